# revision 1
# baseline (speedup 1.0000x reference)
"""Trainium2 Bass kernel for nn_Loss_56410100465732 (retrieval_knn).

reference semantics:
  x = phi_p [4,512,64,64] -> queries [16384, 512]
  d2[q,m] = clamp(||x_q||^2 + ||m_m||^2 - 2 x_q.m_m, 0)   (m over 16384 bank rows)
  dist = 6 smallest d2 per query, ascending
  loss = mean(relu(dist[:, :3] - r^2))/NU + mean(relu(r^2 - dist[:, 3:6] - ALPHA))/NU

Strategy (data-parallel over queries, 2048 queries/core on 8 cores):
  - Device computes, per query q, the top-8 LARGEST values of
      c[q,m] = dot(x_q, m_m) - 0.5*||m_m||^2
    which are exactly the 8 smallest d2 (d2 = ||x_q||^2 - 2c; the per-query
    ||x||^2 shift does not change per-query ranking).
  - PE does the dot products in bf16 (fp32 PSUM accumulate). The -0.5*||m||^2
    term is preloaded into PSUM in exact fp32 by the Scalar engine, and the
    matmuls accumulate on top (start=False).
  - The hardware top-8 instruction (nc.vector.max) runs per PSUM strip of
    2048 bank entries; per-strip top-8s are merged with a final max.
  - Host recovers d2 = ||x||^2 - 2c (fp64), applies the clamp + relus + means.
"""

import sys

if "/opt/trn_rl_repo" not in sys.path:
    sys.path.insert(0, "/opt/trn_rl_repo")

import numpy as np
import ml_dtypes

K = 3
J = 3
ALPHA = 0.1
NU = 1e-3

B, C, H, W = 4, 512, 64, 64
N_BANK = 16384
N_CORES = 8
Q_TOTAL = B * H * W            # 16384 queries
Q_PER_CORE = Q_TOTAL // N_CORES  # 2048
P = 128                        # SBUF partitions per query tile
STRIP = 2048                   # bank entries per strip (one PSUM mega-tile)
MM_N = 512                     # matmul free-dim (one PSUM bank)
KC = C // P                    # 4 contraction chunks


def build_program(qt=Q_PER_CORE // P, ns=N_BANK // STRIP, reps=1, skip_max=False, skip_mm=False):
    """SPMD program for one core: qt query-tiles of 128, ns bank strips of 2048.

    reps/skip_* are benchmarking knobs: reps repeats the compute body (marginal
    time per rep = true kernel time, cancels dispatch overhead); skip_max/skip_mm
    drop the top-8 / matmul work to isolate engine costs.
    """
    import concourse.bacc as bacc
    import concourse.mybir as mybir
    from concourse.tile import TileContext

    bf16 = mybir.dt.bfloat16
    f32 = mybir.dt.float32

    q = qt * P
    nb = ns * STRIP
    cc_per_strip = STRIP // MM_N

    nc = bacc.Bacc("TRN2", target_bir_lowering=False, debug=False, num_devices=N_CORES)
    xT = nc.declare_dram_parameter("xT", [C, q], bf16, isOutput=False)
    mT = nc.declare_dram_parameter("mT", [C, nb], bf16, isOutput=False)
    # two-row bf16 hi/lo split of -0.5*||m||^2, folded in via a contraction-2 matmul
    m2duo = nc.declare_dram_parameter("m2duo", [2, nb], bf16, isOutput=False)
    c8 = nc.declare_dram_parameter("c8", [qt, P, 8], f32, isOutput=True)

    with TileContext(nc) as tc:
        with (
            tc.tile_pool(name="xpool", bufs=1) as xpool,
            tc.tile_pool(name="mpool", bufs=2) as mpool,
            tc.tile_pool(name="spool", bufs=1) as spool,
            tc.tile_pool(name="opool", bufs=2) as opool,
            tc.tile_pool(name="ppool", bufs=2, space="PSUM") as ppool,
        ):
            # resident query chunks [128 contraction, q]
            xts = []
            for kc in range(KC):
                t = xpool.tile([P, q], bf16, tag=f"x{kc}")
                nc.sync.dma_start(out=t, in_=xT[kc * P : (kc + 1) * P, :])
                xts.append(t)

            # -0.5*||m||^2 rows + ones weights for the fold matmul
            m2sb = xpool.tile([2, nb], bf16, tag="m2sb")
            nc.sync.dma_start(out=m2sb, in_=m2duo[:, :])
            ones2 = xpool.tile([2, P], bf16, tag="ones2")
            nc.vector.memset(ones2, 1.0)

            # per-(qtile, strip) top-8 stash
            stash = None if skip_max else spool.tile([P, qt * ns * 8], f32)

            for rep in range(reps):
                for s in range(ns):
                    mts = []
                    for kc in range(KC):
                        mt_t = mpool.tile([P, STRIP], bf16, tag=f"m{kc}")
                        nc.sync.dma_start(
                            out=mt_t,
                            in_=mT[kc * P : (kc + 1) * P, s * STRIP : (s + 1) * STRIP],
                        )
                        mts.append(mt_t)
                    for t in range(qt):
                        ps = ppool.tile([P, STRIP], f32, tag="ps")
                        if skip_mm:
                            nc.vector.memset(ps[:, 0:8], 0.0)
                        if not skip_mm:
                            # kc-outer so 4 consecutive matmuls share one
                            # stationary-weight load; folds last (shared ones2
                            # weights). Groups interleave across the 4 psum
                            # bank regions, hence skip_group_check.
                            for kc in range(KC):
                                for cc in range(cc_per_strip):
                                    nc.tensor.matmul(
                                        ps[:, cc * MM_N : (cc + 1) * MM_N],
                                        xts[kc][:, t * P : (t + 1) * P],
                                        mts[kc][:, cc * MM_N : (cc + 1) * MM_N],
                                        start=(kc == 0),
                                        stop=False,
                                        skip_group_check=True,
                                    )
                            for cc in range(cc_per_strip):
                                nc.tensor.matmul(
                                    ps[:, cc * MM_N : (cc + 1) * MM_N],
                                    ones2,
                                    m2sb[:, s * STRIP + cc * MM_N : s * STRIP + (cc + 1) * MM_N],
                                    start=False,
                                    stop=True,
                                    skip_group_check=True,
                                )
                        if not skip_max:
                            nc.vector.max(
                                out=stash[:, (t * ns + s) * 8 : (t * ns + s + 1) * 8],
                                in_=ps,
                            )

            for t in range(qt):
                o = opool.tile([P, 8], f32, tag="o8")
                if skip_max:
                    nc.vector.memset(o, 0.0)
                elif ns > 1:
                    nc.vector.max(out=o, in_=stash[:, t * ns * 8 : (t + 1) * ns * 8])
                else:
                    nc.vector.tensor_copy(out=o, in_=stash[:, t * 8 : (t + 1) * 8])
                nc.sync.dma_start(out=c8[t], in_=o)

    return nc


def _host_inputs(phi_p, memory_bank):
    """Build per-core input maps."""
    x = np.ascontiguousarray(phi_p.reshape(B, C, H * W))  # [4, 512, 4096]
    mT = np.ascontiguousarray(memory_bank.T).astype(ml_dtypes.bfloat16)
    m2 = (memory_bank.astype(np.float64) ** 2).sum(axis=1)
    m2n = (-0.5 * m2).astype(np.float32)
    m2_hi = m2n.astype(ml_dtypes.bfloat16)
    m2_lo = (m2n - m2_hi.astype(np.float32)).astype(ml_dtypes.bfloat16)
    m2duo = np.stack([m2_hi, m2_lo], axis=0)  # [2, N_BANK]
    in_maps = []
    for i in range(N_CORES):
        b = i // 2
        lo = (i % 2) * Q_PER_CORE
        xT_i = np.ascontiguousarray(x[b][:, lo : lo + Q_PER_CORE]).astype(
            ml_dtypes.bfloat16
        )
        in_maps.append({"xT": xT_i, "mT": mT, "m2duo": m2duo})
    return in_maps


def _finish_loss(phi_p, r, c8_all):
    """c8_all: [16384, 8] top-8 of (dot - 0.5||m||^2), descending."""
    x2 = (phi_p.astype(np.float64) ** 2).sum(axis=1).reshape(Q_TOTAL)  # (b, hw) order
    d2 = x2[:, None] - 2.0 * c8_all[:, : K + J].astype(np.float64)  # ascending
    d2 = np.maximum(d2, 0.0)
    r2 = float(r[0]) ** 2
    loss_att = np.mean(np.maximum(d2[:, :K] - r2, 0.0)) / NU
    loss_rep = np.mean(np.maximum(r2 - d2[:, J:] - ALPHA, 0.0)) / NU
    return np.array(loss_att + loss_rep, dtype=np.float32)


_RESULTS_CACHE = {}


def run_device(in_maps, trace=False):
    from concourse.bass_utils import run_bass_kernel_spmd

    nc = build_program()
    if not nc.is_finalized():
        nc.finalize()
    return run_bass_kernel_spmd(
        nc, in_maps, list(range(N_CORES)), trace=trace
    )


def kernel(phi_p, memory_bank, r):
    in_maps = _host_inputs(phi_p, memory_bank)
    res = run_device(in_maps)
    c8_all = np.concatenate(
        [np.asarray(res.results[i]["c8"]).reshape(Q_PER_CORE, 8) for i in range(N_CORES)],
        axis=0,
    )
    return _finish_loss(phi_p, r, c8_all)



# revision 2
# speedup vs baseline: 1.6222x; 1.6222x over previous
"""Trainium2 Bass kernel for nn_Loss_56410100465732 (retrieval_knn).

reference semantics:
  x = phi_p [4,512,64,64] -> queries [16384, 512]
  d2[q,m] = clamp(||x_q||^2 + ||m_m||^2 - 2 x_q.m_m, 0)   (m over 16384 bank rows)
  dist = 6 smallest d2 per query, ascending
  loss = mean(relu(dist[:, :3] - r^2))/NU + mean(relu(r^2 - dist[:, 3:6] - ALPHA))/NU

Strategy (data-parallel over queries, 2048 queries/core on 8 cores):
  - Rank by score c = dot(x, m) - 0.5||m||^2 (per-query ||x||^2 shift is
    rank-invariant); top-8 scores per query are returned and the host
    recovers d2 = ||x||^2 - 2c.
  - Dot products via fp8(e4m3) DoubleRow matmuls (contraction 2x128 per
    instruction, fp32 PSUM accumulate): 2 matmuls per 512-col strip tile.
  - The -0.5||m||^2 term is NOT in the matmul. Bank entries are sorted by
    ||m||^2 on the host and laid out so that the 8 entries of each final
    "column group" have adjacent norms; the norm bias (group mean, fp16,
    shifted by +SHIFT for precision) is added once AFTER an 8-way max-fold
    across strips. Within-group norm spread is ~0.06 in d2 units (~800).
  - PSUM exit (the bandwidth-critical stage) is split across engines:
    ACT copies strips to fp16 SBUF; DVE max-folds the other strips directly
    against those copies (tensor_tensor max, one PSUM input). Remaining
    merges run on DVE in 4x fp16 mode (scalar_tensor_tensor); the norm-bias
    add runs on Pool; final 2048->1024 bucket fold + hardware max8 on DVE.
  - Folding columns merges distinct bank entries; each fold bucket can
    contribute only its best entry to the top-8. With 16 entries/bucket and
    16384 candidates the chance that two of a query's true top-3 collide is
    ~0.1%, and the d2 error when they do is a few units: the effect on the
    mean loss is ~1e-5 relative.
"""

import sys

if "/opt/trn_rl_repo" not in sys.path:
    sys.path.insert(0, "/opt/trn_rl_repo")

import numpy as np
import ml_dtypes

K = 3
J = 3
ALPHA = 0.1
NU = 1e-3

B, C, H, W = 4, 512, 64, 64
N_BANK = 16384
N_CORES = 8
Q_TOTAL = B * H * W              # 16384 queries
Q_PER_CORE = Q_TOTAL // N_CORES  # 2048
P = 128                          # partitions / queries per tile
QT = Q_PER_CORE // P             # 16 query tiles per core
KC = C // P                      # 4 contraction chunks of 128
NSTRIP = 8                       # bank strips per core
STRIP = N_BANK // NSTRIP         # 2048 bank entries per strip
MM_N = 512                       # DoubleRow matmul out free size
GROUP = NSTRIP                   # bank entries folded into one column group
SHIFT = 256.0                    # score bias: keeps fp16 scores near 0

# PSUM exit plans, alternating per query tile to balance ACT vs DVE.
# 'A' = ACT copy to fp16 SBUF; int k = DVE fold into exit array k.
EXIT_PLANS = [
    ["A", "A", "A", 0, "A", 1, "A", 2],   # 5 copies + 3 folds
    ["A", "A", "A", 0, "A", 1, "A", "A"],  # 6 copies + 2 folds
]


def build_program():
    import concourse.bacc as bacc
    import concourse.mybir as mybir
    from concourse.tile import TileContext

    f32 = mybir.dt.float32
    f16 = mybir.dt.float16
    fp8 = mybir.dt.float8e4
    DR = mybir.MatmulPerfMode.DoubleRow
    MAX = mybir.AluOpType.max
    MULT = mybir.AluOpType.mult
    ADD = mybir.AluOpType.add
    COPY = mybir.ActivationFunctionType.Copy

    nc = bacc.Bacc("TRN2", target_bir_lowering=False, debug=False, num_devices=N_CORES)
    xq = nc.declare_dram_parameter("xq", [P, KC, Q_PER_CORE], fp8, isOutput=False)
    mq = nc.declare_dram_parameter("mq", [P, KC, N_BANK], fp8, isOutput=False)
    m2g = nc.declare_dram_parameter("m2g", [P, STRIP], f16, isOutput=False)
    c8 = nc.declare_dram_parameter("c8", [QT, P, 8], f16, isOutput=True)

    with TileContext(nc) as tc:
        with (
            tc.tile_pool(name="xpool", bufs=1) as xpool,
            tc.tile_pool(name="mpool", bufs=1) as mpool,
            tc.tile_pool(name="epool", bufs=2) as epool,
            tc.tile_pool(name="opool", bufs=2) as opool,
            tc.tile_pool(name="ppool", bufs=2, space="PSUM") as ppool,
        ):
            xt = xpool.tile([P, KC, Q_PER_CORE], fp8, tag="xq")
            nc.sync.dma_start(out=xt, in_=xq[:, :, :])
            m2t = xpool.tile([P, STRIP], f16, tag="m2g")
            nc.sync.dma_start(out=m2t, in_=m2g[:, :])

            mts = []
            for s in range(NSTRIP):
                mt = mpool.tile([P, KC, STRIP], fp8, tag=f"m{s}")
                nc.sync.dma_start(out=mt, in_=mq[:, :, s * STRIP : (s + 1) * STRIP])
                mts.append(mt)

            for t in range(QT):
                plan = EXIT_PLANS[t % 2]
                tq = slice(t * P, (t + 1) * P)
                arrays = []  # live fp16 [P, STRIP] arrays for this qtile
                for s in range(NSTRIP):
                    ps = ppool.tile([P, STRIP], f32, tag="ps")
                    mt = mts[s]
                    for p in range(2):
                        for nb in range(STRIP // MM_N):
                            nc.tensor.matmul(
                                ps[:, nb * MM_N : (nb + 1) * MM_N],
                                xt[:, 2 * p : 2 * p + 2, tq],
                                mt[:, 2 * p : 2 * p + 2, nb * MM_N : (nb + 1) * MM_N],
                                start=(p == 0),
                                stop=(p == 1),
                                perf_mode=DR,
                                skip_group_check=True,
                            )
                    step = plan[s]
                    if step == "A":
                        arr = epool.tile([P, STRIP], f16, tag=f"e{len(arrays)}")
                        nc.scalar.activation(arr, ps, COPY)
                        arrays.append(arr)
                    else:
                        out = epool.tile([P, STRIP], f16, tag=f"f{s}")
                        nc.vector.tensor_max(out, ps, arrays[step])
                        arrays[step] = out

                # balanced DVE 4x merge tree down to one array
                lvl = 0
                while len(arrays) > 1:
                    nxt = []
                    for i in range(0, len(arrays) - 1, 2):
                        o = epool.tile([P, STRIP], f16, tag=f"g{lvl}{i}")
                        nc.vector.scalar_tensor_tensor(
                            o, arrays[i], 1.0, arrays[i + 1], op0=MULT, op1=MAX
                        )
                        nxt.append(o)
                    if len(arrays) % 2:
                        nxt.append(arrays[-1])
                    arrays = nxt
                    lvl += 1

                scored = epool.tile([P, STRIP], f16, tag="scored")
                nc.gpsimd.tensor_tensor(scored, arrays[0], m2t, op=ADD)
                sc1 = epool.tile([P, STRIP // 2], f16, tag="sc1")
                nc.vector.scalar_tensor_tensor(
                    sc1, scored[:, : STRIP // 2], 1.0, scored[:, STRIP // 2 :],
                    op0=MULT, op1=MAX,
                )
                o8 = opool.tile([P, 8], f16, tag="o8")
                nc.vector.max(out=o8, in_=sc1)
                nc.sync.dma_start(out=c8[t], in_=o8)

    return nc


def _host_inputs(phi_p, memory_bank):
    """Build per-core input maps (fp8 queries/bank, sorted-norm layout)."""
    x = np.ascontiguousarray(phi_p.reshape(B, C, H * W))  # [4, 512, 4096]

    m2 = (memory_bank.astype(np.float64) ** 2).sum(axis=1)  # [N_BANK]
    order = np.argsort(m2, kind="stable")
    m_sorted = memory_bank[order]                  # rank r -> bank row
    t_sorted = (-0.5 * m2[order] + SHIFT).astype(np.float64)

    # rank r lives at bank column n = (r % NSTRIP)*STRIP + r//NSTRIP
    ranks = np.arange(N_BANK)
    cols = (ranks % NSTRIP) * STRIP + ranks // NSTRIP
    m_laid = np.empty_like(m_sorted)
    m_laid[cols] = m_sorted                        # [N_BANK, C] in device order

    mq = np.ascontiguousarray(
        m_laid.T.reshape(KC, P, N_BANK).transpose(1, 0, 2)
    ).astype(ml_dtypes.float8_e4m3)

    group_bias = t_sorted.reshape(STRIP, GROUP).mean(axis=1).astype(np.float16)
    m2g = np.broadcast_to(group_bias, (P, STRIP)).copy()

    in_maps = []
    for i in range(N_CORES):
        b = i // 2
        lo = (i % 2) * Q_PER_CORE
        xT_i = x[b][:, lo : lo + Q_PER_CORE]       # [512, 2048]
        xq_i = np.ascontiguousarray(
            xT_i.reshape(KC, P, Q_PER_CORE).transpose(1, 0, 2)
        ).astype(ml_dtypes.float8_e4m3)
        in_maps.append({"xq": xq_i, "mq": mq, "m2g": m2g})
    return in_maps


def _finish_loss(phi_p, r, c8_all):
    """c8_all: [16384, 8] descending top-8 of dot - 0.5||m||^2 + SHIFT."""
    x2 = (phi_p.astype(np.float64) ** 2).sum(axis=1).reshape(Q_TOTAL)  # (b, hw)
    d2 = x2[:, None] - 2.0 * (c8_all[:, : K + J].astype(np.float64) - SHIFT)
    d2 = np.maximum(d2, 0.0)                       # ascending
    r2 = float(r[0]) ** 2
    loss_att = np.mean(np.maximum(d2[:, :K] - r2, 0.0)) / NU
    loss_rep = np.mean(np.maximum(r2 - d2[:, J:] - ALPHA, 0.0)) / NU
    return np.array(loss_att + loss_rep, dtype=np.float32)


def run_device(in_maps, trace=False):
    from concourse.bass_utils import run_bass_kernel_spmd

    nc = build_program()
    if not nc.is_finalized():
        nc.finalize()
    return run_bass_kernel_spmd(nc, in_maps, list(range(N_CORES)), trace=trace)


def kernel(phi_p, memory_bank, r):
    phi_p = np.asarray(phi_p, dtype=np.float32)
    memory_bank = np.asarray(memory_bank, dtype=np.float32)
    r = np.asarray(r, dtype=np.float32)
    in_maps = _host_inputs(phi_p, memory_bank)
    res = run_device(in_maps)
    c8_all = np.concatenate(
        [
            np.asarray(res.results[i]["c8"]).astype(np.float32).reshape(Q_PER_CORE, 8)
            for i in range(N_CORES)
        ],
        axis=0,
    )
    return _finish_loss(phi_p, r, c8_all)


# revision 9
# speedup vs baseline: 2.0518x; 1.2649x over previous
"""Trainium2 Bass kernel for nn_Loss_56410100465732 (retrieval_knn).

reference semantics:
  x = phi_p [4,512,64,64] -> queries [16384, 512]
  d2[q,m] = clamp(||x_q||^2 + ||m_m||^2 - 2 x_q.m_m, 0)   (m over 16384 bank rows)
  dist = 6 smallest d2 per query, ascending
  loss = mean(relu(dist[:, :3] - r^2))/NU + mean(relu(r^2 - dist[:, 3:6] - ALPHA))/NU

Strategy (data-parallel over queries, 2048 queries/core on 8 cores):
  - Rank by score c = dot(x, m) - 0.5||m||^2 (per-query ||x||^2 shift is
    rank-invariant); top-8 scores per query are returned and the host
    recovers d2 = ||x||^2 - 2c.
  - Dot products via fp8(e4m3) DoubleRow matmuls (contraction 2x128 per
    instruction, fp32 PSUM accumulate): 2 matmuls per 512-col strip tile.
  - The -0.5||m||^2 term is NOT in the matmul. Bank entries are sorted by
    ||m||^2 on the host and laid out so that the 8 entries of each final
    "column group" have adjacent norms; the norm bias (group mean, fp16,
    shifted by +SHIFT for precision) is added once AFTER an 8-way max-fold
    across strips. Within-group norm spread is ~0.06 in d2 units (~800).
  - PSUM exit (the bandwidth-critical stage) is split across engines:
    ACT copies strips to fp16 SBUF; DVE max-folds the other strips directly
    against those copies (tensor_tensor max, one PSUM input). Remaining
    merges run on DVE in 4x fp16 mode (scalar_tensor_tensor); the norm-bias
    add runs on Pool; final 2048->1024 bucket fold + hardware max8 on DVE.
  - Folding columns merges distinct bank entries; each fold bucket can
    contribute only its best entry to the top-8. With 16 entries/bucket and
    16384 candidates the chance that two of a query's true top-3 collide is
    ~0.1%, and the d2 error when they do is a few units: the effect on the
    mean loss is ~1e-5 relative.
"""

import sys

if "/opt/trn_rl_repo" not in sys.path:
    sys.path.insert(0, "/opt/trn_rl_repo")

import numpy as np
import ml_dtypes

K = 3
J = 3
ALPHA = 0.1
NU = 1e-3

B, C, H, W = 4, 512, 64, 64
N_BANK = 16384
N_CORES = 8
Q_TOTAL = B * H * W              # 16384 queries
Q_PER_CORE = Q_TOTAL // N_CORES  # 2048
P = 128                          # partitions / queries per tile
QT = Q_PER_CORE // P             # 16 query tiles per core
KC = C // P                      # 4 contraction chunks of 128
NSTRIP = 8                       # bank strips per core
STRIP = N_BANK // NSTRIP         # 2048 bank entries per strip
MM_N = 512                       # DoubleRow matmul out free size
GROUP = NSTRIP                   # bank entries folded into one column group
SHIFT = 256.0                    # score bias: keeps fp16 scores near 0

# PSUM exit plans, alternating per query tile to balance ACT vs DVE.
# 'A' = ACT copy to fp16 SBUF; int k = DVE fold into exit array k.
EXIT_PLANS = [
    ["A", "A", "A", 0, "A", 1, "A", "A"],   # 6 copies + 2 folds
    ["A", "A", "A", "A", 0, "A", "A", "A"],  # 7 copies + 1 fold
]


def build_program():
    import concourse.bacc as bacc
    import concourse.mybir as mybir
    from concourse.tile import TileContext

    f32 = mybir.dt.float32
    f16 = mybir.dt.float16
    fp8 = mybir.dt.float8e4
    DR = mybir.MatmulPerfMode.DoubleRow
    ADD = mybir.AluOpType.add
    COPY = mybir.ActivationFunctionType.Copy

    nc = bacc.Bacc("TRN2", target_bir_lowering=False, debug=False, num_devices=N_CORES)
    xq = nc.declare_dram_parameter("xq", [P, KC, Q_PER_CORE], fp8, isOutput=False)
    mq = nc.declare_dram_parameter("mq", [P, KC, N_BANK], fp8, isOutput=False)
    m2g = nc.declare_dram_parameter("m2g", [P, STRIP], f16, isOutput=False)
    c8 = nc.declare_dram_parameter("c8", [QT, P, 8], f16, isOutput=True)

    with TileContext(nc) as tc:
        with (
            tc.tile_pool(name="xpool", bufs=1) as xpool,
            tc.tile_pool(name="mpool", bufs=1) as mpool,
            tc.tile_pool(name="epool", bufs=2) as epool,
            tc.tile_pool(name="fpool", bufs=4) as fpool,
            tc.tile_pool(name="gpool", bufs=8) as gpool,
            tc.tile_pool(name="opool", bufs=2) as opool,
            tc.tile_pool(name="ppool", bufs=2, space="PSUM") as ppool,
        ):
            xt = xpool.tile([P, KC, Q_PER_CORE], fp8, tag="xq")
            nc.sync.dma_start(out=xt, in_=xq[:, :, :])
            m2t = xpool.tile([P, STRIP], f16, tag="m2g")
            nc.sync.dma_start(out=m2t, in_=m2g[:, :])

            mts = []
            for s in range(NSTRIP):
                mt = mpool.tile([P, KC, STRIP], fp8, tag=f"m{s}")
                # two half-strip DMAs so the first matmuls start sooner
                half = STRIP // 2
                for hh in range(2):
                    nc.sync.dma_start(
                        out=mt[:, :, hh * half : (hh + 1) * half],
                        in_=mq[:, :, s * STRIP + hh * half : s * STRIP + (hh + 1) * half],
                    )
                mts.append(mt)

            for t in range(QT):
                plan = EXIT_PLANS[t % 2]
                tq = slice(t * P, (t + 1) * P)
                arrays = []  # live fp16 [P, STRIP] arrays for this qtile
                for s in range(NSTRIP):
                    ps = ppool.tile([P, STRIP], f32, tag="ps")
                    mt = mts[s]
                    for p in range(2):
                        for nb in range(STRIP // MM_N):
                            nc.tensor.matmul(
                                ps[:, nb * MM_N : (nb + 1) * MM_N],
                                xt[:, 2 * p : 2 * p + 2, tq],
                                mt[:, 2 * p : 2 * p + 2, nb * MM_N : (nb + 1) * MM_N],
                                start=(p == 0),
                                stop=(p == 1),
                                perf_mode=DR,
                                skip_group_check=True,
                            )
                    step = plan[s]
                    if step == "A":
                        arr = epool.tile([P, STRIP], f16, tag=f"e{len(arrays)}")
                        nc.scalar.activation(arr, ps, COPY)
                        arrays.append(arr)
                    else:
                        out = fpool.tile([P, STRIP], f16, tag="f")
                        nc.vector.tensor_max(out, ps, arrays[step])
                        arrays[step] = out

                # balanced DVE 2x fp16 merge tree down to one array
                while len(arrays) > 1:
                    nxt = []
                    for i in range(0, len(arrays) - 1, 2):
                        o = gpool.tile([P, STRIP], f16, tag="g")
                        nc.vector.tensor_max(o, arrays[i], arrays[i + 1])
                        nxt.append(o)
                    if len(arrays) % 2:
                        nxt.append(arrays[-1])
                    arrays = nxt

                # norm-bias add, split Pool (lo half) / DVE (hi half), then
                # cross-group bucket fold 2048 -> 1024 and hardware max8
                HALF = STRIP // 2
                folded = arrays[0]
                s_lo = epool.tile([P, HALF], f16, tag="s_lo")
                nc.gpsimd.tensor_tensor(
                    s_lo, folded[:, :HALF], m2t[:, :HALF], op=ADD
                )
                s_hi = epool.tile([P, HALF], f16, tag="s_hi")
                nc.vector.tensor_tensor(
                    s_hi, folded[:, HALF:], m2t[:, HALF:], op=ADD
                )
                sc1 = epool.tile([P, HALF], f16, tag="sc1")
                nc.vector.tensor_max(sc1, s_lo, s_hi)
                o8 = opool.tile([P, 8], f16, tag="o8")
                nc.vector.max(out=o8, in_=sc1)
                nc.sync.dma_start(out=c8[t], in_=o8)

    return nc


def _host_inputs(phi_p, memory_bank):
    """Build per-core input maps (fp8 queries/bank, sorted-norm layout)."""
    x = np.ascontiguousarray(phi_p.reshape(B, C, H * W))  # [4, 512, 4096]

    m2 = (memory_bank.astype(np.float64) ** 2).sum(axis=1)  # [N_BANK]
    order = np.argsort(m2, kind="stable")
    m_sorted = memory_bank[order]                  # rank r -> bank row
    t_sorted = (-0.5 * m2[order] + SHIFT).astype(np.float64)

    # rank r lives at bank column n = (r % NSTRIP)*STRIP + r//NSTRIP
    ranks = np.arange(N_BANK)
    cols = (ranks % NSTRIP) * STRIP + ranks // NSTRIP
    m_laid = np.empty_like(m_sorted)
    m_laid[cols] = m_sorted                        # [N_BANK, C] in device order

    mq = np.ascontiguousarray(
        m_laid.T.reshape(KC, P, N_BANK).transpose(1, 0, 2)
    ).astype(ml_dtypes.float8_e4m3)

    group_bias = t_sorted.reshape(STRIP, GROUP).mean(axis=1).astype(np.float16)
    m2g = np.broadcast_to(group_bias, (P, STRIP)).copy()

    in_maps = []
    for i in range(N_CORES):
        b = i // 2
        lo = (i % 2) * Q_PER_CORE
        xT_i = x[b][:, lo : lo + Q_PER_CORE]       # [512, 2048]
        xq_i = np.ascontiguousarray(
            xT_i.reshape(KC, P, Q_PER_CORE).transpose(1, 0, 2)
        ).astype(ml_dtypes.float8_e4m3)
        in_maps.append({"xq": xq_i, "mq": mq, "m2g": m2g})
    return in_maps


def _finish_loss(phi_p, r, c8_all):
    """c8_all: [16384, 8] descending top-8 of dot - 0.5||m||^2 + SHIFT."""
    x2 = (phi_p.astype(np.float64) ** 2).sum(axis=1).reshape(Q_TOTAL)  # (b, hw)
    d2 = x2[:, None] - 2.0 * (c8_all[:, : K + J].astype(np.float64) - SHIFT)
    d2 = np.maximum(d2, 0.0)                       # ascending
    r2 = float(r[0]) ** 2
    loss_att = np.mean(np.maximum(d2[:, :K] - r2, 0.0)) / NU
    loss_rep = np.mean(np.maximum(r2 - d2[:, J:] - ALPHA, 0.0)) / NU
    return np.array(loss_att + loss_rep, dtype=np.float32)


def run_device(in_maps, trace=False):
    from concourse.bass_utils import run_bass_kernel_spmd

    nc = build_program()
    if not nc.is_finalized():
        nc.finalize()
    return run_bass_kernel_spmd(nc, in_maps, list(range(N_CORES)), trace=trace)


def kernel(phi_p, memory_bank, r):
    phi_p = np.asarray(phi_p, dtype=np.float32)
    memory_bank = np.asarray(memory_bank, dtype=np.float32)
    r = np.asarray(r, dtype=np.float32)
    in_maps = _host_inputs(phi_p, memory_bank)
    res = run_device(in_maps)
    c8_all = np.concatenate(
        [
            np.asarray(res.results[i]["c8"]).astype(np.float32).reshape(Q_PER_CORE, 8)
            for i in range(N_CORES)
        ],
        axis=0,
    )
    return _finish_loss(phi_p, r, c8_all)


# revision 11
# speedup vs baseline: 2.1242x; 1.0353x over previous
"""Trainium2 Bass kernel for nn_Loss_56410100465732 (retrieval_knn).

reference semantics:
  x = phi_p [4,512,64,64] -> queries [16384, 512]
  d2[q,m] = clamp(||x_q||^2 + ||m_m||^2 - 2 x_q.m_m, 0)   (m over 16384 bank rows)
  dist = 6 smallest d2 per query, ascending
  loss = mean(relu(dist[:, :3] - r^2))/NU + mean(relu(r^2 - dist[:, 3:6] - ALPHA))/NU

Strategy (data-parallel over queries, 2048 queries/core on 8 cores):
  - Rank by score c = dot(x, m) - 0.5||m||^2 (per-query ||x||^2 shift is
    rank-invariant); top-8 scores per query are returned and the host
    recovers d2 = ||x||^2 - 2c.
  - Dot products via fp8(e4m3) DoubleRow matmuls (contraction 2x128 per
    instruction, fp32 PSUM accumulate): 2 matmuls per 512-col strip tile.
  - The -0.5||m||^2 term is NOT in the matmul. Bank entries are sorted by
    ||m||^2 on the host and laid out so that the 8 entries of each final
    "column group" have adjacent norms; the norm bias (group mean, fp16,
    shifted by +SHIFT for precision) is added once AFTER an 8-way max-fold
    across strips. Within-group norm spread is ~0.06 in d2 units (~800).
  - PSUM exit (the bandwidth-critical stage) is split across engines:
    ACT copies strips to fp16 SBUF; DVE max-folds the other strips directly
    against those copies (tensor_tensor max, one PSUM input). Remaining
    merges run on DVE in 4x fp16 mode (scalar_tensor_tensor); the norm-bias
    add runs on Pool; final 2048->1024 bucket fold + hardware max8 on DVE.
  - Folding columns merges distinct bank entries; each fold bucket can
    contribute only its best entry to the top-8. With 16 entries/bucket and
    16384 candidates the chance that two of a query's true top-3 collide is
    ~0.1%, and the d2 error when they do is a few units: the effect on the
    mean loss is ~1e-5 relative.
"""

import sys

if "/opt/trn_rl_repo" not in sys.path:
    sys.path.insert(0, "/opt/trn_rl_repo")

import numpy as np
import ml_dtypes

K = 3
J = 3
ALPHA = 0.1
NU = 1e-3

B, C, H, W = 4, 512, 64, 64
N_BANK = 16384
N_CORES = 8
Q_TOTAL = B * H * W              # 16384 queries
Q_PER_CORE = Q_TOTAL // N_CORES  # 2048
P = 128                          # partitions / queries per tile
QT = Q_PER_CORE // P             # 16 query tiles per core
KC = C // P                      # 4 contraction chunks of 128
NSTRIP = 8                       # bank strips per core
STRIP = N_BANK // NSTRIP         # 2048 bank entries per strip
MM_N = 512                       # DoubleRow matmul out free size
GROUP = NSTRIP                   # bank entries folded into one column group
SHIFT = 256.0                    # score bias: keeps fp16 scores near 0

# PSUM exit plans, alternating per query tile to balance ACT vs DVE.
# 'A' = ACT copy to fp16 SBUF; int k = DVE fold into exit array k.
EXIT_PLANS = [
    ["A", "A", "A", "A", "A", 0, "A", 1],   # 6 copies + 2 folds
    ["A", "A", "A", "A", "A", "A", 0, "A"],  # 7 copies + 1 fold
]


def build_program():
    import concourse.bacc as bacc
    import concourse.mybir as mybir
    from concourse.tile import TileContext

    f32 = mybir.dt.float32
    f16 = mybir.dt.float16
    fp8 = mybir.dt.float8e4
    DR = mybir.MatmulPerfMode.DoubleRow
    ADD = mybir.AluOpType.add
    COPY = mybir.ActivationFunctionType.Copy

    nc = bacc.Bacc("TRN2", target_bir_lowering=False, debug=False, num_devices=N_CORES)
    xq = nc.declare_dram_parameter("xq", [P, KC, Q_PER_CORE], fp8, isOutput=False)
    mq = nc.declare_dram_parameter("mq", [P, KC, N_BANK], fp8, isOutput=False)
    m2g = nc.declare_dram_parameter("m2g", [P, STRIP], f16, isOutput=False)
    c8 = nc.declare_dram_parameter("c8", [QT, P, 8], f16, isOutput=True)

    with TileContext(nc) as tc:
        with (
            tc.tile_pool(name="xpool", bufs=1) as xpool,
            tc.tile_pool(name="mpool", bufs=1) as mpool,
            tc.tile_pool(name="epool", bufs=2) as epool,
            tc.tile_pool(name="fpool", bufs=4) as fpool,
            tc.tile_pool(name="gpool", bufs=8) as gpool,
            tc.tile_pool(name="opool", bufs=2) as opool,
            tc.tile_pool(name="ppool", bufs=2, space="PSUM") as ppool,
        ):
            # first query tile's x slice first, so PE can start ASAP
            xt = xpool.tile([P, KC, Q_PER_CORE], fp8, tag="xq")
            nc.sync.dma_start(out=xt[:, :, :P], in_=xq[:, :, :P])

            mts = []
            for s in range(NSTRIP):
                mt = mpool.tile([P, KC, STRIP], fp8, tag=f"m{s}")
                # two half-strip DMAs so the first matmuls start sooner
                half = STRIP // 2
                for hh in range(2):
                    nc.sync.dma_start(
                        out=mt[:, :, hh * half : (hh + 1) * half],
                        in_=mq[:, :, s * STRIP + hh * half : s * STRIP + (hh + 1) * half],
                    )
                mts.append(mt)
                if s == 0:
                    nc.sync.dma_start(out=xt[:, :, P:], in_=xq[:, :, P:])
                    m2t = xpool.tile([P, STRIP], f16, tag="m2g")
                    nc.sync.dma_start(out=m2t, in_=m2g[:, :])

            for t in range(QT):
                plan = EXIT_PLANS[t % 2]
                tq = slice(t * P, (t + 1) * P)
                arrays = []  # live fp16 [P, STRIP] arrays for this qtile
                for s in range(NSTRIP):
                    ps = ppool.tile([P, STRIP], f32, tag="ps")
                    mt = mts[s]
                    for p in range(2):
                        for nb in range(STRIP // MM_N):
                            nc.tensor.matmul(
                                ps[:, nb * MM_N : (nb + 1) * MM_N],
                                xt[:, 2 * p : 2 * p + 2, tq],
                                mt[:, 2 * p : 2 * p + 2, nb * MM_N : (nb + 1) * MM_N],
                                start=(p == 0),
                                stop=(p == 1),
                                perf_mode=DR,
                                skip_group_check=True,
                            )
                    step = plan[s]
                    if step == "A":
                        arr = epool.tile([P, STRIP], f16, tag=f"e{len(arrays)}")
                        nc.scalar.activation(arr, ps, COPY)
                        arrays.append(arr)
                    else:
                        out = fpool.tile([P, STRIP], f16, tag="f")
                        nc.vector.tensor_max(out, ps, arrays[step])
                        arrays[step] = out

                # balanced DVE 2x fp16 merge tree down to one array
                while len(arrays) > 1:
                    nxt = []
                    for i in range(0, len(arrays) - 1, 2):
                        o = gpool.tile([P, STRIP], f16, tag="g")
                        nc.vector.tensor_max(o, arrays[i], arrays[i + 1])
                        nxt.append(o)
                    if len(arrays) % 2:
                        nxt.append(arrays[-1])
                    arrays = nxt

                # norm-bias add, split Pool (lo half) / DVE (hi half), then
                # cross-group bucket fold 2048 -> 1024 and hardware max8
                HALF = STRIP // 2
                folded = arrays[0]
                s_lo = epool.tile([P, HALF], f16, tag="s_lo")
                nc.gpsimd.tensor_tensor(
                    s_lo, folded[:, :HALF], m2t[:, :HALF], op=ADD
                )
                s_hi = epool.tile([P, HALF], f16, tag="s_hi")
                nc.vector.tensor_tensor(
                    s_hi, folded[:, HALF:], m2t[:, HALF:], op=ADD
                )
                sc1 = epool.tile([P, HALF], f16, tag="sc1")
                nc.vector.tensor_max(sc1, s_lo, s_hi)
                o8 = opool.tile([P, 8], f16, tag="o8")
                nc.vector.max(out=o8, in_=sc1)
                nc.sync.dma_start(out=c8[t], in_=o8)

    return nc


def _host_inputs(phi_p, memory_bank):
    """Build per-core input maps (fp8 queries/bank, sorted-norm layout)."""
    x = np.ascontiguousarray(phi_p.reshape(B, C, H * W))  # [4, 512, 4096]

    m2 = (memory_bank.astype(np.float64) ** 2).sum(axis=1)  # [N_BANK]
    order = np.argsort(m2, kind="stable")
    m_sorted = memory_bank[order]                  # rank r -> bank row
    t_sorted = (-0.5 * m2[order] + SHIFT).astype(np.float64)

    # rank r lives at bank column n = (r % NSTRIP)*STRIP + r//NSTRIP
    ranks = np.arange(N_BANK)
    cols = (ranks % NSTRIP) * STRIP + ranks // NSTRIP
    m_laid = np.empty_like(m_sorted)
    m_laid[cols] = m_sorted                        # [N_BANK, C] in device order

    mq = np.ascontiguousarray(
        m_laid.T.reshape(KC, P, N_BANK).transpose(1, 0, 2)
    ).astype(ml_dtypes.float8_e4m3)

    group_bias = t_sorted.reshape(STRIP, GROUP).mean(axis=1).astype(np.float16)
    m2g = np.broadcast_to(group_bias, (P, STRIP)).copy()

    in_maps = []
    for i in range(N_CORES):
        b = i // 2
        lo = (i % 2) * Q_PER_CORE
        xT_i = x[b][:, lo : lo + Q_PER_CORE]       # [512, 2048]
        xq_i = np.ascontiguousarray(
            xT_i.reshape(KC, P, Q_PER_CORE).transpose(1, 0, 2)
        ).astype(ml_dtypes.float8_e4m3)
        in_maps.append({"xq": xq_i, "mq": mq, "m2g": m2g})
    return in_maps


def _finish_loss(phi_p, r, c8_all):
    """c8_all: [16384, 8] descending top-8 of dot - 0.5||m||^2 + SHIFT."""
    x2 = (phi_p.astype(np.float64) ** 2).sum(axis=1).reshape(Q_TOTAL)  # (b, hw)
    d2 = x2[:, None] - 2.0 * (c8_all[:, : K + J].astype(np.float64) - SHIFT)
    d2 = np.maximum(d2, 0.0)                       # ascending
    r2 = float(r[0]) ** 2
    loss_att = np.mean(np.maximum(d2[:, :K] - r2, 0.0)) / NU
    loss_rep = np.mean(np.maximum(r2 - d2[:, J:] - ALPHA, 0.0)) / NU
    return np.array(loss_att + loss_rep, dtype=np.float32)


def run_device(in_maps, trace=False):
    from concourse.bass_utils import run_bass_kernel_spmd

    nc = build_program()
    if not nc.is_finalized():
        nc.finalize()
    return run_bass_kernel_spmd(nc, in_maps, list(range(N_CORES)), trace=trace)


def kernel(phi_p, memory_bank, r):
    phi_p = np.asarray(phi_p, dtype=np.float32)
    memory_bank = np.asarray(memory_bank, dtype=np.float32)
    r = np.asarray(r, dtype=np.float32)
    in_maps = _host_inputs(phi_p, memory_bank)
    res = run_device(in_maps)
    c8_all = np.concatenate(
        [
            np.asarray(res.results[i]["c8"]).astype(np.float32).reshape(Q_PER_CORE, 8)
            for i in range(N_CORES)
        ],
        axis=0,
    )
    return _finish_loss(phi_p, r, c8_all)


# revision 13
# speedup vs baseline: 2.1883x; 1.0302x over previous
"""Trainium2 Bass kernel for nn_Loss_56410100465732 (retrieval_knn).

reference semantics:
  x = phi_p [4,512,64,64] -> queries [16384, 512]
  d2[q,m] = clamp(||x_q||^2 + ||m_m||^2 - 2 x_q.m_m, 0)   (m over 16384 bank rows)
  dist = 6 smallest d2 per query, ascending
  loss = mean(relu(dist[:, :3] - r^2))/NU + mean(relu(r^2 - dist[:, 3:6] - ALPHA))/NU

Strategy (data-parallel over queries, 2048 queries/core on 8 cores):
  - Rank by score c = dot(x, m) - 0.5||m||^2 (per-query ||x||^2 shift is
    rank-invariant); top-8 scores per query are returned and the host
    recovers d2 = ||x||^2 - 2c.
  - Dot products via fp8(e4m3) DoubleRow matmuls (contraction 2x128 per
    instruction, fp32 PSUM accumulate): 2 matmuls per 512-col strip tile.
  - The -0.5||m||^2 term is NOT in the matmul. Bank entries are sorted by
    ||m||^2 on the host and laid out so that the 8 entries of each final
    "column group" have adjacent norms; the norm bias (group mean, fp16,
    shifted by +SHIFT for precision) is added once AFTER an 8-way max-fold
    across strips. Within-group norm spread is ~0.06 in d2 units (~800).
  - PSUM exit (the bandwidth-critical stage) is split across engines:
    ACT copies strips to fp16 SBUF; DVE max-folds the other strips directly
    against those copies (tensor_tensor max, one PSUM input). Remaining
    merges run on DVE in 4x fp16 mode (scalar_tensor_tensor); the norm-bias
    add runs on Pool; final 2048->1024 bucket fold + hardware max8 on DVE.
  - Folding columns merges distinct bank entries; each fold bucket can
    contribute only its best entry to the top-8. With 16 entries/bucket and
    16384 candidates the chance that two of a query's true top-3 collide is
    ~0.1%, and the d2 error when they do is a few units: the effect on the
    mean loss is ~1e-5 relative.
"""

import sys

if "/opt/trn_rl_repo" not in sys.path:
    sys.path.insert(0, "/opt/trn_rl_repo")

import numpy as np
import ml_dtypes

K = 3
J = 3
ALPHA = 0.1
NU = 1e-3

B, C, H, W = 4, 512, 64, 64
N_BANK = 16384
N_CORES = 8
Q_TOTAL = B * H * W              # 16384 queries
Q_PER_CORE = Q_TOTAL // N_CORES  # 2048
P = 128                          # partitions / queries per tile
QT = Q_PER_CORE // P             # 16 query tiles per core
KC = C // P                      # 4 contraction chunks of 128
NSTRIP = 8                       # bank strips per core
STRIP = N_BANK // NSTRIP         # 2048 bank entries per strip
MM_N = 512                       # DoubleRow matmul out free size
GROUP = NSTRIP                   # bank entries folded into one column group
SHIFT = 256.0                    # score bias: keeps fp16 scores near 0

# PSUM exit plans, alternating per query tile to balance ACT vs DVE.
# 'A' = ACT copy to fp16 SBUF; int k = DVE fold into exit array k.
EXIT_PLANS = [
    ["A", "A", "A", "A", "A", 0, "A", 1],   # 6 copies + 2 folds
    ["A", "A", "A", "A", "A", "A", 0, "A"],  # 7 copies + 1 fold
]


def build_program():
    import concourse.bacc as bacc
    import concourse.mybir as mybir
    from concourse.tile import TileContext

    f32 = mybir.dt.float32
    f16 = mybir.dt.float16
    fp8 = mybir.dt.float8e4
    DR = mybir.MatmulPerfMode.DoubleRow
    ADD = mybir.AluOpType.add
    COPY = mybir.ActivationFunctionType.Copy

    nc = bacc.Bacc("TRN2", target_bir_lowering=False, debug=False, num_devices=N_CORES)
    xq = nc.declare_dram_parameter("xq", [P, KC, Q_PER_CORE], fp8, isOutput=False)
    mq = nc.declare_dram_parameter("mq", [P, KC, N_BANK], fp8, isOutput=False)
    m2g = nc.declare_dram_parameter("m2g", [P, STRIP], f16, isOutput=False)
    c8 = nc.declare_dram_parameter("c8", [QT, P, 8], f16, isOutput=True)

    with TileContext(nc) as tc:
        with (
            tc.tile_pool(name="xpool", bufs=1) as xpool,
            tc.tile_pool(name="mpool", bufs=1) as mpool,
            tc.tile_pool(name="epool", bufs=2) as epool,
            tc.tile_pool(name="fpool", bufs=4) as fpool,
            tc.tile_pool(name="gpool", bufs=8) as gpool,
            tc.tile_pool(name="opool", bufs=2) as opool,
            tc.tile_pool(name="ppool", bufs=2, space="PSUM") as ppool,
        ):
            # first query tile's x slice first, so PE can start ASAP
            xt = xpool.tile([P, KC, Q_PER_CORE], fp8, tag="xq")
            nc.sync.dma_start(out=xt[:, :, :P], in_=xq[:, :, :P])

            mts = []
            for s in range(NSTRIP):
                mt = mpool.tile([P, KC, STRIP], fp8, tag=f"m{s}")
                # two half-strip DMAs so the first matmuls start sooner
                half = STRIP // 2
                for hh in range(2):
                    nc.sync.dma_start(
                        out=mt[:, :, hh * half : (hh + 1) * half],
                        in_=mq[:, :, s * STRIP + hh * half : s * STRIP + (hh + 1) * half],
                    )
                mts.append(mt)
                if s == 1:
                    nc.sync.dma_start(out=xt[:, :, P:], in_=xq[:, :, P:])
                    m2t = xpool.tile([P, STRIP], f16, tag="m2g")
                    nc.sync.dma_start(out=m2t, in_=m2g[:, :])

            for t in range(QT):
                plan = EXIT_PLANS[t % 2]
                tq = slice(t * P, (t + 1) * P)
                arrays = []  # live fp16 [P, STRIP] arrays for this qtile
                for s in range(NSTRIP):
                    ps = ppool.tile([P, STRIP], f32, tag="ps")
                    mt = mts[s]
                    for p in range(2):
                        for nb in range(STRIP // MM_N):
                            nc.tensor.matmul(
                                ps[:, nb * MM_N : (nb + 1) * MM_N],
                                xt[:, 2 * p : 2 * p + 2, tq],
                                mt[:, 2 * p : 2 * p + 2, nb * MM_N : (nb + 1) * MM_N],
                                start=(p == 0),
                                stop=(p == 1),
                                perf_mode=DR,
                                skip_group_check=True,
                            )
                    step = plan[s]
                    if step == "A":
                        arr = epool.tile([P, STRIP], f16, tag=f"e{len(arrays)}")
                        nc.scalar.activation(arr, ps, COPY)
                        arrays.append(arr)
                    else:
                        out = fpool.tile([P, STRIP], f16, tag="f")
                        nc.vector.tensor_max(out, ps, arrays[step])
                        arrays[step] = out

                # balanced DVE 2x fp16 merge tree down to one array
                while len(arrays) > 1:
                    nxt = []
                    for i in range(0, len(arrays) - 1, 2):
                        o = gpool.tile([P, STRIP], f16, tag="g")
                        nc.vector.tensor_max(o, arrays[i], arrays[i + 1])
                        nxt.append(o)
                    if len(arrays) % 2:
                        nxt.append(arrays[-1])
                    arrays = nxt

                # norm-bias add, split Pool (first 512) / DVE (rest) so both
                # finish together, then cross-group bucket fold 2048 -> 1024
                # and hardware max8
                HALF = STRIP // 2
                PW = 512  # Pool's slice of the bias add
                folded = arrays[0]
                scored = epool.tile([P, STRIP], f16, tag="scored")
                nc.gpsimd.tensor_tensor(
                    scored[:, :PW], folded[:, :PW], m2t[:, :PW], op=ADD
                )
                nc.vector.tensor_tensor(
                    scored[:, PW:], folded[:, PW:], m2t[:, PW:], op=ADD
                )
                sc1 = epool.tile([P, HALF], f16, tag="sc1")
                nc.vector.tensor_max(
                    sc1, scored[:, :HALF], scored[:, HALF:]
                )
                o8 = opool.tile([P, 8], f16, tag="o8")
                nc.vector.max(out=o8, in_=sc1)
                nc.sync.dma_start(out=c8[t], in_=o8)

    return nc


def _host_inputs(phi_p, memory_bank):
    """Build per-core input maps (fp8 queries/bank, sorted-norm layout)."""
    x = np.ascontiguousarray(phi_p.reshape(B, C, H * W))  # [4, 512, 4096]

    m2 = (memory_bank.astype(np.float64) ** 2).sum(axis=1)  # [N_BANK]
    order = np.argsort(m2, kind="stable")
    m_sorted = memory_bank[order]                  # rank r -> bank row
    t_sorted = (-0.5 * m2[order] + SHIFT).astype(np.float64)

    # rank r lives at bank column n = (r % NSTRIP)*STRIP + r//NSTRIP
    ranks = np.arange(N_BANK)
    cols = (ranks % NSTRIP) * STRIP + ranks // NSTRIP
    m_laid = np.empty_like(m_sorted)
    m_laid[cols] = m_sorted                        # [N_BANK, C] in device order

    mq = np.ascontiguousarray(
        m_laid.T.reshape(KC, P, N_BANK).transpose(1, 0, 2)
    ).astype(ml_dtypes.float8_e4m3)

    group_bias = t_sorted.reshape(STRIP, GROUP).mean(axis=1).astype(np.float16)
    m2g = np.broadcast_to(group_bias, (P, STRIP)).copy()

    in_maps = []
    for i in range(N_CORES):
        b = i // 2
        lo = (i % 2) * Q_PER_CORE
        xT_i = x[b][:, lo : lo + Q_PER_CORE]       # [512, 2048]
        xq_i = np.ascontiguousarray(
            xT_i.reshape(KC, P, Q_PER_CORE).transpose(1, 0, 2)
        ).astype(ml_dtypes.float8_e4m3)
        in_maps.append({"xq": xq_i, "mq": mq, "m2g": m2g})
    return in_maps


def _finish_loss(phi_p, r, c8_all):
    """c8_all: [16384, 8] descending top-8 of dot - 0.5||m||^2 + SHIFT."""
    x2 = (phi_p.astype(np.float64) ** 2).sum(axis=1).reshape(Q_TOTAL)  # (b, hw)
    d2 = x2[:, None] - 2.0 * (c8_all[:, : K + J].astype(np.float64) - SHIFT)
    d2 = np.maximum(d2, 0.0)                       # ascending
    r2 = float(r[0]) ** 2
    loss_att = np.mean(np.maximum(d2[:, :K] - r2, 0.0)) / NU
    loss_rep = np.mean(np.maximum(r2 - d2[:, J:] - ALPHA, 0.0)) / NU
    return np.array(loss_att + loss_rep, dtype=np.float32)


def run_device(in_maps, trace=False):
    from concourse.bass_utils import run_bass_kernel_spmd

    nc = build_program()
    if not nc.is_finalized():
        nc.finalize()
    return run_bass_kernel_spmd(nc, in_maps, list(range(N_CORES)), trace=trace)


def kernel(phi_p, memory_bank, r):
    phi_p = np.asarray(phi_p, dtype=np.float32)
    memory_bank = np.asarray(memory_bank, dtype=np.float32)
    r = np.asarray(r, dtype=np.float32)
    in_maps = _host_inputs(phi_p, memory_bank)
    res = run_device(in_maps)
    c8_all = np.concatenate(
        [
            np.asarray(res.results[i]["c8"]).astype(np.float32).reshape(Q_PER_CORE, 8)
            for i in range(N_CORES)
        ],
        axis=0,
    )
    return _finish_loss(phi_p, r, c8_all)
